# revision 71
# baseline (speedup 1.0000x reference)
"""Trainium2 Bass kernel for nn_CNNGenerator (frame CNN + FC + window-sum + FC).

Key algebraic facts exploited (validated vs the reference):
  * softmax over a size-1 axis == 1.0, so the whole attention_conv stack is
    dead code; the bmm reduces to an 8-wide sliding-window sum of ffc.
  * The per-window stride-2 conv stack collapses into global conv streams:
    an "interior" stream g{1,2,3} and a "left-edge" stream e{1,2,3} per
    layer, plus a 2-tap combine (z).  Per t:
      g1[s] = b1 + sum_k W1k x[s+k-8]          e1[t] = b1 + W11 x[t-7] + W12 x[t-6]
      g2[s] = b2 + V0 G1[s-2] + V1 G1[s] + V2 G1[s+2]
      e2[t] = b2 + V1 E1[t] + V2 G1[t+2]
      g3[s] = b3 + U0 G2[s-4] + U1 G2[s] + U2 G2[s+4]
      e3[t] = b3 + U1 E2[t] + U2 G2[t+4]
      z[t]  = b4 + T1 E3[t] + T2 G3[t+8]
    (capitals = leaky-activated streams), then fc1/fc2/fc3+tanh,
    ws[t] = sum_{d=-3..4} ffc[t+d], out = clip(fcw @ ws, 0, 1).

Sharding: pure data parallel, 2 batch elements per core on 8 cores.
On-chip layout: time axis split in 4 chunks of 2048; 32-channel streams pack
4 chunks x 32ch on the 128 partitions, 64-channel streams pack 2 chunks x 64ch
(two tiles).  Weights are host-packed into block-diagonal lhsT matrices.

Perf structure (TimelineSim 79.6us vs 113.7us baseline; rel err 3.6e-3):
  * All matmul operands bf16 (1 cycle/row at any width; fp32 PSUM accum);
    output tensor bf16 (values clipped to [0,1], host casts back to f32).
  * Two-half column wavefront with per-layer staggered boundaries
    (G1@1034 > G2@1032 > G3@1028 > rest@1020 > ws@1016, halo-cascaded) so
    each pass's half-0 never needs a producer's half-1 columns; passes are
    emitted layer-major with halves adjacent, keeping every engine fed.
  * G1/E1 fused in one psum pass: E1 = taps{1,2}, evac, then tap{0}
    accumulates on top for G1 (saves 2 of 5 layer-1 taps on PE).
  * fc1 layer uses the relu decomposition leaky(x) = a*x + (1-a)*relu(x):
    H1 streams hold relu(fw1@H + fb1) (1-inst evac, rotated DVE/Act),
    and A2 gains a composed a*fw2@fw1 tap over H (slot 17).  This moves
    ~14us of evacuation off the Activation engine for +3.4us of PE.
  * Other evacuations: Act activation (Prelu/Tanh, fused bias+leaky,
    1024-col psum groups); halo tails (<32 col) on DVE.  GPSIMD cannot
    read PSUM (BIR constraint), so Pool only handles memsets.
  * window-sum as a 2-level adder tree (S2 = 4-sums on DVE) + 2-tap final
    matmul (fcw@S2[t-3] + fcw@S2[t+1]); tail chunks pipeline under the
    second half's conv work, final clip on DVE, chunked output DMA.
    Half-0 final-fc chunks (<=512 col) run from the 1-bank halo-tail psum
    ring so they never contend with FFC's groups in the main ring.
"""
import sys

if '/opt/trn_rl_repo' not in sys.path:
    sys.path.insert(0, '/opt/trn_rl_repo')

import numpy as np
import ml_dtypes

B, C, T = 16, 29, 8192
NCORES = 8
BPC = B // NCORES          # batch per core
Tc = T // 4                # time chunk
Tcp = Tc + 22              # per-chunk dram row width (10 left pad, 12 right)
HL = 16                    # left halo: tile col u <-> global idx c*Tc + u - HL
W = Tc + 40                # per-batch stream tile width
W2 = BPC * W
NSLOT = 18                 # 128-col lhsT slots in the weight pack
NWCOL = NSLOT * 128

_PROG = {}
PS_GROUP = 1024
SC_BUFS = 3
WS_BUFS = 2
USE_BF16 = True
BF16 = ml_dtypes.bfloat16
NPDT = BF16 if USE_BF16 else np.float32

# evacuation engine per stream: 'act' | 'dve' | 'pool'
# (E1 has its bias folded into the layer-1 matmul via the ones rows, so its
#  vector evac needs only 2 instructions)
# H1 streams hold relu(fw1@H + fb1) (not leaky!) -- 1-inst vector evac; the
# linear alpha-path of fc1's leaky is folded into A2's slot-17 matmul tap.
# engine per (pass, half, batch) group of the H1 relu evacuations --
# 16 groups rotated across act/dve/pool to keep all three busy
RELU_SEQ = ['dve', 'act'] * 8
RELU = {'H1a': True, 'H1b': True, 'H1c': True, 'H1d': True}
EVAC = {
    'G1': 'act', 'E1': 'act', 'G2': 'act', 'E2': 'act',
    'G3a': 'act', 'G3b': 'act', 'E3a': 'act', 'E3b': 'act',
    'Ha': 'act', 'Hb': 'act',
    'H1a': 'act', 'H1b': 'act', 'H1c': 'act', 'H1d': 'act',
    'A2a': 'act', 'A2b': 'act', 'FFC': 'act',
}

PASS_ORDER = ['G1', 'E1', 'G2', 'E2', 'G3a', 'G3b', 'E3a', 'E3b',
              'Ha', 'Hb', 'H1a', 'H1b', 'H1c', 'H1d', 'A2a', 'A2b', 'FFC']


def _blockdiag(blocks):
    k = sum(b.shape[0] for b in blocks)
    m = sum(b.shape[1] for b in blocks)
    out = np.zeros((k, m), np.float32)
    i = j = 0
    for b in blocks:
        out[i:i + b.shape[0], j:j + b.shape[1]] = b
        i += b.shape[0]
        j += b.shape[1]
    return out


def _pack_weights(inp):
    wp = np.zeros((128, NWCOL), np.float32)

    def put(slot, mat):
        wp[:mat.shape[0], slot * 128: slot * 128 + mat.shape[1]] = mat

    w1 = inp['w1'].astype(np.float32)  # [32, 29, 3]
    b1 = inp['b1'].astype(np.float32)
    # slots 0-2: g1 taps: blockdiag4 of [30, 32]: 29 in-ch rows + bias row
    for k in range(3):
        blk = np.zeros((30, 32), np.float32)
        blk[:29] = w1[:, :, k].T
        if k == 1:
            blk[29] = b1
        put(k, _blockdiag([blk] * 4))
    w2 = inp['w2'].astype(np.float32)
    for k in range(3):  # slots 3-5
        put(3 + k, _blockdiag([w2[:, :, k].T.astype(np.float32)] * 4))
    w3 = inp['w3'].astype(np.float32)
    for k in range(3):  # slots 6-8: [64, 128], duplicated at rows 64:128 so the
        # pair-1 matmuls (rhs base partition 64) see the same base
        blk = _blockdiag([w3[:, :, k].T.astype(np.float32)] * 2)
        put(6 + k, np.concatenate([blk, blk], axis=0))
    w4 = inp['w4'].astype(np.float32)
    for j in (1, 2):    # slots 9-10: [128, 128]
        put(9 + j - 1, _blockdiag([w4[:, :, j].T.astype(np.float32)] * 2))
    fw1t = inp['fw1'].T.astype(np.float32)
    put(11, np.concatenate([fw1t, fw1t], axis=0))               # [64,128] x2 rows
    # fc2 / fc3 as M=128 with zero column-halves: psum accumulation composes
    # the two chunk-halves onto partitions 0:64 / 64:128 without col-tiling.
    # fc2 via the relu decomposition of fc1's leaky (alpha=0.02):
    #   A2pre = a*fw2@fw1 @ H + (1-a)*fw2 @ relu(fw1@H + fb1) + (a*fw2@fb1+fb2)
    fw2t = (1.0 - 0.02) * inp['fw2'].T.astype(np.float32)   # [128, 64]
    z64 = np.zeros_like(fw2t)
    put(12, np.concatenate([fw2t, z64], axis=1))    # fc2_lo [128, 128]
    put(15, np.concatenate([z64, fw2t], axis=1))    # fc2_hi
    m2 = 0.02 * (inp['fw2'].astype(np.float32) @ inp['fw1'].astype(np.float32))
    put(17, _blockdiag([m2.T] * 2))                 # [128, 128]
    fw3t = _blockdiag([inp['fw3'].T.astype(np.float32)] * 2)       # [128, 64]
    z64b = np.zeros_like(fw3t)
    put(13, np.concatenate([fw3t, z64b], axis=1))   # fc3_lo
    put(16, np.concatenate([z64b, fw3t], axis=1))   # fc3_hi
    put(14, _blockdiag([inp['fcw'].T.astype(np.float32)] * 4))     # [128, 64]
    return wp.astype(NPDT)


def _pack_biases(inp):
    bp = np.zeros((128, 8), np.float32)
    bp[:, 0] = np.tile(inp['b2'], 4)
    bp[:, 1] = np.tile(inp['b3'], 2)
    bp[:, 2] = np.tile(inp['b4'], 2)
    bp[:, 3] = inp['fb1']
    bp[:, 4] = np.tile(0.02 * (inp['fw2'].astype(np.float32)
                               @ inp['fb1'].astype(np.float32))
                       + inp['fb2'].astype(np.float32), 2)
    bp[:, 5] = np.tile(inp['fb3'], 4)
    return bp


def _split(lo, hi, step=512):
    return [(a, min(a + step, hi)) for a in range(lo, hi, step)]


def _groups(lo, hi):
    """[lo,hi) -> up to two 1024-col main groups + small tail."""
    out = []
    a = lo
    while hi - a > 32:
        out.append((a, a + PS_GROUP, False))
        a += PS_GROUP
    if a < hi:
        out.append((a, hi, True))
    return out


def _build_program(reps=1):
    import concourse.bacc as bacc
    import concourse.mybir as mybir
    import concourse.tile as tile

    F32 = mybir.dt.float32
    BF = mybir.dt.bfloat16 if USE_BF16 else mybir.dt.float32
    AF = mybir.ActivationFunctionType
    OP = mybir.AluOpType

    nc = bacc.Bacc("TRN2", target_bir_lowering=False, debug=False)
    x_d = nc.dram_tensor("x", [BPC, 4, C + 1, Tcp], BF, kind="ExternalInput").ap()
    w_d = nc.dram_tensor("wpack", [128, NWCOL], BF, kind="ExternalInput").ap()
    bias_d = nc.dram_tensor("biases", [128, 8], F32, kind="ExternalInput").ap()
    o_d = nc.dram_tensor("out", [BPC, 4, 16, Tc], BF, kind="ExternalOutput").ap()

    with tile.TileContext(nc) as tc:
        with tc.tile_pool(name="wp", bufs=1) as wpool, \
             tc.tile_pool(name="xp", bufs=1) as xpool, \
             tc.tile_pool(name="yp", bufs=1) as ypool, \
             tc.tile_pool(name="st", bufs=(1 if USE_BF16 else 7)) as spool, \
             tc.tile_pool(name="sc", bufs=SC_BUFS) as scpool, \
             tc.tile_pool(name="ws", bufs=WS_BUFS) as wspool, \
             tc.tile_pool(name="ps", bufs=3, space="PSUM") as ppool, \
             tc.tile_pool(name="pt", bufs=2, space="PSUM") as ptpool:

            wsb = wpool.tile([128, NWCOL], BF, tag="w")
            bsb = wpool.tile([128, 8], F32, tag="b")
            X = xpool.tile([120, W2], BF, tag="x")

            # ---- input/weight DMA, ordered so layer-1 work starts early ----
            nc.sync.dma_start(out=wsb[:, 0:3 * 128], in_=w_d[:, 0:3 * 128])
            XSPL = 1032
            for b in range(BPC):
                nc.sync.dma_start(out=X[:, b * W + 6: b * W + 6 + XSPL],
                                  in_=x_d[b, :, :, 0:XSPL])
            nc.sync.dma_start(out=bsb[:], in_=bias_d[:])
            for b in range(BPC):
                nc.sync.dma_start(out=X[:, b * W + 6 + XSPL: b * W + 6 + Tcp],
                                  in_=x_d[b, :, :, XSPL:Tcp])
            nc.sync.dma_start(out=wsb[:, 3 * 128:11 * 128],
                              in_=w_d[:, 3 * 128:11 * 128])
            nc.sync.dma_start(out=wsb[:, 11 * 128:], in_=w_d[:, 11 * 128:])

            def lhsT(slot, k=128, m=128, base=0):
                return wsb[base:base + k, slot * 128: slot * 128 + m]

            def bias(i):
                return bsb[:, i:i + 1]

            for _rep in range(reps):
                _emit_body(nc, tc, mybir, F32, BF, AF, OP, wsb, lhsT, bias,
                           xpool, ypool, spool, scpool, wspool,
                           ppool, ptpool, X, o_d)
    nc.finalize()
    return nc


def _emit_body(nc, tc, mybir, F32, BF, AF, OP, wsb, lhsT, bias,
               xpool, ypool, spool, scpool, wspool, ppool, ptpool, X, o_d):
    ST = lambda nm: spool.tile([128, W2], BF,  # noqa: E731
                               tag=("st_" + nm) if USE_BF16 else "st",
                               name=nm)
    F32R = mybir.dt.float32r

    def mm(ap):
        return ap if USE_BF16 else ap.bitcast(F32R)

    def evac_act(func, bias_ap, alpha):
        def f(ps, ot, b=0):
            nc.scalar.activation(ot, ps, func, bias=bias_ap, scale=1.0,
                                 alpha=alpha).annotate(_lbl[0] + ':evac')
        return f

    def evac_vec(eng, bias_ap, alpha):
        """leaky(p + b) = max(p + b, alpha*(p + b)) on DVE/Pool.
        bias_ap None: bias came in via the matmul; 2-instruction form."""
        def f(ps, ot, b=0):
            n = ps.shape[-1]
            t1 = scpool.tile([128, PS_GROUP], BF, tag="t1" + eng)
            nc_e = nc.vector if eng == 'dve' else nc.gpsimd
            if bias_ap is None:
                nc_e.tensor_scalar(t1[:, 0:n], ps, alpha, None,
                                   OP.mult).annotate(_lbl[0] + ':evac')
                nc_e.tensor_tensor(ot, ps, t1[:, 0:n],
                                   OP.max).annotate(_lbl[0] + ':evac')
            else:
                t2 = scpool.tile([128, PS_GROUP], BF, tag="t2" + eng)
                nc_e.tensor_scalar(t1[:, 0:n], ps, bias_ap, alpha,
                                   OP.add, OP.mult).annotate(_lbl[0] + ':evac')
                nc_e.tensor_scalar(t2[:, 0:n], ps, bias_ap, None,
                                   OP.add).annotate(_lbl[0] + ':evac')
                nc_e.tensor_tensor(ot, t1[:, 0:n], t2[:, 0:n],
                                   OP.max).annotate(_lbl[0] + ':evac')
        return f

    def evac_vec_tail(eng, bias_ap, alpha):
        """2-dim small-tail variant of evac_vec."""
        def f(ps, ot, b=0):
            n = ps.shape[-1]
            t1 = scpool.tile([128, 32], BF, tag="t1tl")
            nc_e = nc.vector if eng == 'dve' else nc.gpsimd
            if bias_ap is None:
                nc_e.tensor_scalar(t1[:, 0:n], ps, alpha, None, OP.mult)
                nc_e.tensor_tensor(ot, ps, t1[:, 0:n], OP.max)
            else:
                t2 = scpool.tile([128, 32], BF, tag="t2tl")
                nc_e.tensor_scalar(t1[:, 0:n], ps, bias_ap, alpha,
                                   OP.add, OP.mult)
                nc_e.tensor_scalar(t2[:, 0:n], ps, bias_ap, None, OP.add)
                nc_e.tensor_tensor(ot, t1[:, 0:n], t2[:, 0:n], OP.max)
        return f

    _relu_ctr = [0]

    def evac_relu(eng, bias_ap):
        def f(ps, ot, b=0):
            if eng == 'rot':
                e = RELU_SEQ[_relu_ctr[0] % len(RELU_SEQ)]
                _relu_ctr[0] += 1
            else:
                e = eng
            if e == 'act':
                nc.scalar.activation(ot, ps, AF.Relu, bias=bias_ap,
                                     scale=1.0).annotate(_lbl[0] + ':evac')
            elif e == 'dp':
                # DVE moves psum->sbuf; idle Pool applies bias+relu in SBUF
                n = ps.shape[-1]
                t1 = scpool.tile([128, PS_GROUP], BF, tag="t1dp")
                nc.vector.tensor_copy(
                    t1[:, 0:n], ps).annotate(_lbl[0] + ':evac')
                nc.gpsimd.tensor_scalar(
                    ot, t1[:, 0:n], bias_ap, 0.0, OP.add,
                    OP.max).annotate(_lbl[0] + ':evac')
            else:
                nc_e = nc.vector if e == 'dve' else nc.gpsimd
                nc_e.tensor_scalar(ot, ps, bias_ap, 0.0, OP.add,
                                   OP.max).annotate(_lbl[0] + ':evac')
        return f

    def mk_evac(name, func, bias_ap, alpha):
        if name in RELU:
            return evac_relu('rot', bias_ap), evac_relu('dve', bias_ap)
        eng = EVAC[name]
        if eng == 'act':
            main = evac_act(func, bias_ap if bias_ap is not None else 0.0,
                            alpha)
        else:
            main = evac_vec(eng, bias_ap, alpha)
        if func == AF.Tanh:
            tailf = main
        else:
            tailf = evac_vec_tail('dve', bias_ap, alpha)
        return main, tailf

    def fused_pass(e_out, e_rng, e_taps, e_evacs,
                   g_out, g_rng, g_taps, g_evacs,
                   shared_taps, half, split, lbl):
        """E and G streams sharing matmul taps: psum group accumulates
        shared_taps over the union range, then e_taps; E evacuates; then
        g_taps accumulate on top (minus E's contribution is NOT needed --
        G = shared + g_taps, E = shared + e_taps, so g_taps must include
        the e_taps' negation?  No: E reads psum after shared+e;
        G reads after shared+e+g, so g_taps are chosen with
        E_contribution included in G (true for these layers where
        G = shared + extra taps and E = shared + e_taps with e_taps==0).
        Here e_taps add E-only terms; G's total must equal
        shared + g_taps + e_taps restricted to... (see call sites: E-only
        taps cover columns also seen by G only when the e-tap rhs stream
        contributes to G as well -- G1/G2/G3 call sites keep e_taps empty
        or e-only columns disjoint from g reads)."""
        lo, hi = min(e_rng[0], g_rng[0]), max(e_rng[1], g_rng[1])
        for b in range(BPC):
            if half == 0:
                gl = [(lo, split, False)]
            else:
                gl = _groups(split, hi)
            for (glo, ghi, is_tail) in gl:
                if is_tail:
                    ps = ptpool.tile([128, 512], F32, tag="pst", name="pst")
                else:
                    ps = ppool.tile([128, PS_GROUP], F32, tag="ps", name="ps")

                def emit_taps(taps, r0, r1, first, last_stop):
                    c0, c1 = max(glo, r0), min(ghi, r1)
                    if c0 >= c1:
                        return first
                    for (lo5, hi5) in _split(c0, c1, 512):
                        n, off = hi5 - lo5, lo5 - glo
                        for i, (lw, rt, rp0, rp1, d) in enumerate(taps):
                            nc.tensor.matmul(
                                ps[0:128, off:off + n], mm(lw),
                                mm(rt[rp0:rp1,
                                      b * W + lo5 + d: b * W + hi5 + d]),
                                start=first and i == 0,
                                stop=(last_stop and i == len(taps) - 1),
                                skip_group_check=True).annotate(lbl)
                    return False

                first = emit_taps(shared_taps, lo, hi, True, False)
                first = emit_taps(e_taps, e_rng[0], e_rng[1], first, True)
                e0, e1 = max(glo, e_rng[0]), min(ghi, e_rng[1])
                if e0 < e1:
                    (e_evacs[1] if is_tail else e_evacs[0])(
                        ps[:, e0 - glo: e1 - glo],
                        e_out[:, b * W + e0: b * W + e1], b)
                emit_taps(g_taps, g_rng[0], g_rng[1], False, True)
                g0, g1_ = max(glo, g_rng[0]), min(ghi, g_rng[1])
                if g0 < g1_:
                    (g_evacs[1] if is_tail else g_evacs[0])(
                        ps[:, g0 - glo: g1_ - glo],
                        g_out[:, b * W + g0: b * W + g1_], b)

    def conv_pass(out_tile, rng, taps, evacs, half=None, split=None):
        """taps: list of (lhsT_ap, rhs_tile, rp0, rp1, delta).
        half 0: cols [lo, split); half 1: [split, hi) in 1024-col groups +
        halo tail.  Splits are staggered across layers so that half-0
        consumers never need half-1 producer columns."""
        main, tailf = evacs
        for b in range(BPC):
            if half is None:
                gl = _groups(rng[0], rng[1])
            elif half == 0:
                gl = [(rng[0], split, False)]
            else:
                gl = _groups(split, rng[1])
            for (glo, ghi, is_tail) in gl:
                gn = ghi - glo
                if is_tail:
                    ps = ptpool.tile([128, 512], F32, tag="pst", name="pst")
                else:
                    ps = ppool.tile([128, PS_GROUP], F32, tag="ps", name="ps")
                spl = _split(glo, ghi, 512)
                for i, (lw, rt, rp0, rp1, d) in enumerate(taps):
                    for (lo, hi) in spl:
                        n, off = hi - lo, lo - glo
                        nc.tensor.matmul(
                            ps[0:128, off:off + n], mm(lw),
                            mm(rt[rp0:rp1, b * W + lo + d: b * W + hi + d]),
                            start=(i == 0),
                            stop=(i == len(taps) - 1)).annotate(_lbl[0])
                (tailf if is_tail else main)(
                    ps[:, 0:gn], out_tile[:, b * W + glo: b * W + ghi], b)

    _lbl = ['?']
    G1, E1, G2, E2 = ST("G1"), ST("E1"), ST("G2"), ST("E2")
    G3 = [ST("G3a"), ST("G3b")]
    E3 = [ST("E3a"), ST("E3b")]
    H = [ST("Ha"), ST("Hb")]
    H1 = [ST("H1a"), ST("H1b"), ST("H1c"), ST("H1d")]
    A2 = [ST("A2a"), ST("A2b")]
    FFC = ST("FFC")

    passes = {
        # E1 = taps{1,2} over X; G1 = E1's psum + tap{0}. One shared psum.
        'GE1': lambda h: fused_pass(
            E1, (13, Tc + 21), [], mk_evac('E1', AF.Prelu, None, 0.02),
            G1, (14, Tc + 34),
            [(lhsT(0, 120), X, 0, 120, -8)],
            mk_evac('G1', AF.Prelu, None, 0.02),
            [(lhsT(k, 120), X, 0, 120, k - 8) for k in (1, 2)],
            h, 1034, 'GE1' + str(h)),
        # shared tap S = V2*G1(+2); E2 = S + V1*E1; G2 = E2psum? no --
        # G2 = S + V0*G1(-2) + V1*G1(0): E2's V1*E1 term is NOT in G2, so
        # E2 must be evacuated from a COPY...  -> keep separate psums:
        # instead share only between G-computation orders (see fused_pass
        # docstring).  E2/G2 fusion invalid; keep plain passes.
        'G2': lambda h: conv_pass(G2, (17, Tc + 33),
            [(lhsT(3 + k), G1, 0, 128, 2 * (k - 1)) for k in range(3)],
            mk_evac('G2', AF.Prelu, bias(0), 0.02), h, 1032),
        'E2': lambda h: conv_pass(E2, (13, Tc + 21),
            [(lhsT(4), E1, 0, 128, 0), (lhsT(5), G1, 0, 128, 2)],
            mk_evac('E2', AF.Prelu, bias(0), 0.02), h, 1020),
    }
    for p in range(2):
        passes['G3' + 'ab'[p]] = (lambda p: lambda h: conv_pass(
            G3[p], (21, Tc + 29),
            [(lhsT(6 + k, 64, base=64 * p), G2,
              64 * p, 64 * p + 64, 4 * (k - 1)) for k in range(3)],
            mk_evac('G3' + 'ab'[p], AF.Prelu, bias(1), 0.2), h, 1028))(p)
        passes['E3' + 'ab'[p]] = (lambda p: lambda h: conv_pass(
            E3[p], (13, Tc + 21),
            [(lhsT(7, 64, base=64 * p), E2, 64 * p, 64 * p + 64, 0),
             (lhsT(8, 64, base=64 * p), G2, 64 * p, 64 * p + 64, 4)],
            mk_evac('E3' + 'ab'[p], AF.Prelu, bias(1), 0.2), h, 1020))(p)
        passes['H' + 'ab'[p]] = (lambda p: lambda h: conv_pass(
            H[p], (13, Tc + 21),
            [(lhsT(9), E3[p], 0, 128, 0), (lhsT(10), G3[p], 0, 128, 8)],
            mk_evac('H' + 'ab'[p], AF.Prelu, bias(2), 0.2), h, 1020))(p)
        passes['A2' + 'ab'[p]] = (lambda p: lambda h: conv_pass(
            A2[p], (13, Tc + 21),
            [(lhsT(17), H[p], 0, 128, 0),
             (lhsT(12), H1[2 * p], 0, 128, 0),
             (lhsT(15), H1[2 * p + 1], 0, 128, 0)],
            mk_evac('A2' + 'ab'[p], AF.Prelu, bias(4), 0.02), h, 1020))(p)
    for cidx in range(4):
        p, hf = cidx // 2, cidx % 2
        passes['H1' + 'abcd'[cidx]] = (lambda p, hf, cidx: lambda h: conv_pass(
            H1[cidx], (13, Tc + 21),
            [(lhsT(11, 64, base=64 * hf), H[p],
              64 * hf, 64 * hf + 64, 0)],
            mk_evac('H1' + 'abcd'[cidx], AF.Prelu, bias(3), 0.02),
            h, 1020))(p, hf, cidx)
    passes['FFC'] = lambda h: conv_pass(FFC, (13, Tc + 21),
        [(lhsT(13), A2[0], 0, 128, 0), (lhsT(16), A2[1], 0, 128, 0)],
        mk_evac('FFC', AF.Tanh, bias(5), 0.0), h, 1020)

    # ---------------- tail: window sum -> final fc -> clip -> DMA, per half
    # ws[t] = sum_{d=-3..4} ffc[t+d] via S2[v] = sum_{j=0..3} F[v+j] (2 DVE
    # tree levels) and a 2-tap final matmul: out = fcw@S2[t-3] + fcw@S2[t+1].
    S1 = ST("S1")
    Y = ypool.tile([128, Tc], BF, tag="y")
    CH_WS = [(HL, 1016), (1016, Tc + HL)]
    CHF = [[(HL, 528), (528, 1016)],
           [(1016, 1540), (1540, Tc + HL)]]

    def tail_half(h):
        # ffc outside the valid t-range [0, T) must read as ZERO in the
        # window sum (reference zero-pads ffc, not just x): chunk0 cols
        # t=-3..-1 (half 0), chunk3 t=T..T+4 (half 1).
        for b in range(BPC):
            if h == 0:
                nc.gpsimd.memset(FFC[0:32, b * W + 13: b * W + 16], 0.0)
            else:
                nc.gpsimd.memset(
                    FFC[96:128, b * W + Tc + 16: b * W + Tc + 21], 0.0)
        u0, u1 = CH_WS[h]
        a0 = u0 - 3              # A[v] = F[v] + F[v+1] over [a0, a1)
        a1 = u1 + 3
        s0 = 13 if h == 0 else 1017   # S2[v] = A[v] + A[v+2] over [s0, s1)
        s1 = u1 + 1
        for b in range(BPC):
            o = b * W
            A = wspool.tile([128, PS_GROUP + 48], BF, tag="wsA")
            nc.vector.tensor_tensor(A[:, 0:a1 - a0],
                                    FFC[:, o + a0: o + a1],
                                    FFC[:, o + a0 + 1: o + a1 + 1], OP.add)
            nc.vector.tensor_tensor(S1[:, o + s0: o + s1],
                                    A[:, s0 - a0: s1 - a0],
                                    A[:, s0 - a0 + 2: s1 - a0 + 2], OP.add)
        for (f0, f1) in CHF[h]:
            fn = f1 - f0
            if h == 0:
                ps = ptpool.tile([128, 512], F32, tag="pst", name="psf")
            else:
                ps = ppool.tile([128, PS_GROUP], F32, tag="ps", name="psf")
            for b in range(BPC):
                for (lo, hi) in _split(f0, f1, 512):
                    for i, d in enumerate((-3, 1)):
                        nc.tensor.matmul(
                            ps[64 * b: 64 * b + 64, lo - f0: hi - f0],
                            mm(lhsT(14, 128, 64)),
                            mm(S1[:, b * W + lo + d: b * W + hi + d]),
                            start=(i == 0), stop=(i == 1),
                            tile_position=(0, 64 * b))
            nc.vector.tensor_scalar(Y[:, f0 - HL: f1 - HL], ps[:, 0:fn],
                                    0.0, 1.0, OP.max, OP.min)
            nc.sync.dma_start(out=o_d[:, :, :, f0 - HL: f1 - HL],
                              in_=Y[:, f0 - HL: f1 - HL])

    for nm in ['GE1', 'G2', 'E2', 'G3a', 'G3b', 'E3a', 'E3b']:
        for h in (0, 1):
            _lbl[0] = nm + str(h)
            passes[nm](h)
    for nm, h in (('Ha', 0), ('Ha', 1), ('H1a', 0), ('H1a', 1),
                  ('H1b', 0), ('H1b', 1), ('Hb', 0), ('Hb', 1),
                  ('A2a', 0), ('A2a', 1), ('H1c', 0), ('H1c', 1),
                  ('H1d', 0), ('H1d', 1), ('A2b', 0), ('FFC', 0),
                  ('A2b', 1)):
        _lbl[0] = nm + str(h)
        passes[nm](h)
    _lbl[0] = 'FFC1'
    passes['FFC'](1)
    _lbl[0] = 'tail0'
    tail_half(0)
    _lbl[0] = 'tail1'
    tail_half(1)


def _get_program(reps=1):
    global _PROG
    if _PROG is None:
        _PROG = {}
    if reps not in _PROG:
        _PROG[reps] = _build_program(reps)
    return _PROG[reps]


def kernel(**inputs):
    from concourse.bass_utils import run_bass_kernel_spmd

    x = np.asarray(inputs['speech_features'], np.float32)
    # chunked + haloed input: xa4[b, c, ch, :] = x[b, ch, c*Tc-10 : c*Tc+Tc+12]
    xpad = np.zeros((B, C + 1, T + 22), np.float32)
    xpad[:, :C, 10:10 + T] = x
    xpad[:, C, :] = 1.0
    xa4 = np.empty((B, 4, C + 1, Tcp), np.float32)
    for c in range(4):
        xa4[:, c] = xpad[:, :, c * Tc: c * Tc + Tcp]
    xa4 = xa4.astype(NPDT)
    wp = _pack_weights({k: np.asarray(v, np.float32) for k, v in inputs.items()
                        if k != 'speech_features'})
    bp = _pack_biases({k: np.asarray(v, np.float32) for k, v in inputs.items()
                       if k != 'speech_features'})
    nc = _get_program()
    in_maps = [{"x": xa4[i * BPC:(i + 1) * BPC], "wpack": wp, "biases": bp}
               for i in range(NCORES)]
    res = run_bass_kernel_spmd(nc, in_maps, core_ids=list(range(NCORES)))
    # out dram layout: [BPC, 4(chunk), 16(cls), Tc] -> [BPC, T, 16]
    outs = [r["out"].transpose(0, 1, 3, 2).reshape(BPC, T, 16)
            for r in res.results]
    return np.ascontiguousarray(np.concatenate(outs, axis=0).astype(np.float32))
